# revision 26
# baseline (speedup 1.0000x reference)
"""Trainium2 Bass kernel: 7x7 VALID conv (expressed as three 7x3 convs).

x: [16, 64, 112, 112] int32 (values 0..7), w1/w2/w3: [128, 64, 7, 3]
(small ints 0..6; w2 only has its middle column nonzero).  The reference
out1[:, :, :, :-2] + out2[:, :, :, 2:-2] + out3[:, :, :, 2:] is exactly a
VALID 7x7 conv with W7 (w1 -> cols 0:3, w2 -> cols 2:5, w3 -> cols 4:7),
output [16, 128, 106, 106] float32.  All arithmetic is small-integer and
exactly representable in bf16/fp8e4m3 with fp32 PSUM accumulation, so the
kernel result is bit-exact vs the reference.

Strategy: data-parallel over batch N across 8 cores (2 images/core).
Direct conv as implicit GEMM on the TensorEngine:
  - SBUF x layout [128, 113*112]: partitions 0:64 = channels (flat H*W,
    one zero pad row), partitions 64:128 = same shifted by one image row
    (+112 elements).  A K=128 matmul therefore contracts 64 channels x 2
    vertical taps (kh, kh+1) at once.
  - fp8 DoubleRow additionally contracts 2 sub-elements per partition via
    a [128, 2, N] rhs AP with pair-stride 224 (= 2 rows): 4 vertical taps
    per matmul -> 14 matmuls per output tile instead of 28.
  - Output tiles are 4 output rows x 112 columns (N=448, only 106 cols
    valid per row); invalid columns are computed and discarded by the
    strided PSUM->DRAM store.
"""

import numpy as np

import concourse.bass as bass
import concourse.mybir as mybir
from concourse import tile
from concourse.ap import AP
from concourse.bacc import Bacc
from concourse.bass_utils import run_bass_kernel_spmd
from concourse.tile import add_dep_helper

N_CORES = 8
IMGS = 2            # images per core
C, O = 64, 128
H = W = 112
HO = WO = 106
FLAT = 113 * 112    # per-image padded flat length (one zero row at the end)
R = 4               # output rows per tile

USE_FP8 = True
TRACE = False  # set True (e.g. from test.py) to capture an NTFF profile

# tap schedules: (kh0, kw) covering kh0 + {0,1} (bf16) or kh0 + {0,1,2,3} (fp8)
TAPS_FP8 = [(kh0, kw) for kw in range(7) for kh0 in (0, 4)]     # 14 taps
TAPS_BF16 = [(kh0, kw) for kw in range(7) for kh0 in (0, 2, 4, 6)]  # 28 taps

GROUPS = [(h0, min(R, HO - h0)) for h0 in range(0, HO, R)]      # 27 groups

_FOLLOW_GROUPS = set()  # {(img, gi)}: debug — trace Tile dep insertion

CHUNK_ROWS = 56  # output rows per store DMA (2 chunks/image)

last_exec_time_ns = None


def _build(use_fp8, imgs=IMGS, groups=GROUPS):
    dt = mybir.dt.float8e4 if use_fp8 else mybir.dt.bfloat16
    taps = TAPS_FP8 if use_fp8 else TAPS_BF16
    wcols = len(taps) * (2 * 128 if use_fp8 else 128)

    # Bacc (not bare Bass): its finalize() legalizes sync for TRN2 —
    # splits >1-wait instructions into EventSemaphore chains and moves
    # matmul waits onto ldweights
    nc = Bacc()
    xp = nc.declare_dram_parameter("xp", [imgs, 128, FLAT], dt, isOutput=False)
    wt = nc.declare_dram_parameter("wt", [128, wcols], dt, isOutput=False)
    out = nc.declare_dram_parameter(
        "out", [imgs, O, HO, WO], mybir.dt.float32, isOutput=True
    )

    NPS = 6  # PSUM banks cycled round-robin (of 8)

    with tile.TileContext(nc) as tc:
        with (
            tc.tile_pool(name="xpool", bufs=2) as xpool,
            tc.tile_pool(name="wpool", bufs=1) as wpool,
            tc.tile_pool(name="opool", bufs=2) as opool,
            tc.tile_pool(name="psum", bufs=1, space="PSUM") as psum_pool,
        ):
            wtile = wpool.tile([128, wcols], dt)
            nc.sync.dma_start(wtile[:], wt[:])
            if use_fp8:
                w4 = wtile.rearrange("p (q i m) -> p q i m", i=2, m=128)
            else:
                w3 = wtile.rearrange("p (q m) -> p q m", m=128)

            # walrus only allows ONE sync-wait on a Matmult.  Tile's slot
            # allocator makes the first matmul of a group that reuses a PSUM
            # bank wait on the bank's release (PE sem AND the DVE copy's sem
            # = 2 waits).  Pre-advance the PE engine's observed DVE/DMA
            # clocks instead: hang sync deps on the PREVIOUS group's last
            # matmuls (0 waits, spare budget); wait emission is clock-aware,
            # so the reusing matmul then only emits its same-engine PE wait.
            slot_free = {}  # psum tag -> copy instruction that freed the slot
            prev_mms = None  # previous group's matmul instructions

            for img in range(imgs):
                xt = xpool.tile([128, FLAT], dt, tag="x")
                xdma = nc.sync.dma_start(xt[:], xp[img])
                xfull = xt[:]
                pdim = list(xfull.ap[0])
                # whole-image staging buffer; per-group slices are written
                # once each, so the DVE copies carry no WAR-on-DMA waits
                ot = opool.tile([128, HO * WO], mybir.dt.float32, tag="ot")
                ot3 = ot.rearrange("p (h w) -> p h w", w=WO)
                done_rows = 0

                for gi, (h0, r) in enumerate(groups):
                    n = r * 112
                    tag = f"ps{gi % NPS}"
                    # full-bank allocation (512 f32 = 2 KB): unaligned PSUM
                    # tiles share banks and start=True's bank-granular clear
                    # then drags in cross-tile deps
                    psb = psum_pool.tile(
                        [128, 512], mybir.dt.float32, tag=tag, bufs=1
                    )
                    ps = psb[:, 0:n]
                    pre_deps = []
                    if tag in slot_free:
                        pre_deps.append(slot_free[tag])
                    if gi == 0:
                        pre_deps.append(xdma.ins)
                    if prev_mms is not None:
                        for k, d in enumerate(pre_deps):
                            add_dep_helper(
                                prev_mms[-1 - k].ins, d,
                                reason="pre-advance PE observed clock",
                            )
                    mms = []
                    for i, (kh0, kw) in enumerate(taps):
                        f0 = (h0 + kh0) * 112 + kw
                        if use_fp8:
                            rhs = AP(
                                xfull.tensor,
                                xfull.offset + f0,
                                [pdim, [224, 2], [1, n]],
                            )
                            mm = nc.tensor.matmul(
                                ps[:],
                                w4[:, i],
                                rhs,
                                start=(i == 0),
                                stop=(i == len(taps) - 1),
                                perf_mode=mybir.MatmulPerfMode.DoubleRow,
                            )
                        else:
                            rhs = AP(
                                xfull.tensor,
                                xfull.offset + f0,
                                [pdim, [1, n]],
                            )
                            mm = nc.tensor.matmul(
                                ps[:],
                                w3[:, i],
                                rhs,
                                start=(i == 0),
                                stop=(i == len(taps) - 1),
                            )
                        mms.append(mm)
                        if _FOLLOW_GROUPS and (img, gi) in _FOLLOW_GROUPS:
                            tile.tile_follow(mm, log_all_deps=True)
                    prev_mms = mms
                    ps3 = ps.rearrange("p (r w) -> p r w", w=112)
                    cp = nc.vector.tensor_copy(ot3[:, h0 : h0 + r, :], ps3[:, :, 0:WO])
                    if _FOLLOW_GROUPS and (img, gi) in _FOLLOW_GROUPS:
                        tile.tile_follow(cp, log_all_deps=True)
                    slot_free[tag] = cp.ins
                    # drain finished rows in chunks so the final DMA tail is
                    # short and each DMA carries a single DVE-sem wait
                    rows_ready = h0 + r
                    last_rows = groups[-1][0] + groups[-1][1]
                    if rows_ready - done_rows >= CHUNK_ROWS or rows_ready == last_rows:
                        # gpsimd (SWDGE) path: the HWDGE direct-2D lowering
                        # only allows one sync wait and image 1's chunks
                        # carry two (DVE data dep + HW queue reuse)
                        nc.gpsimd.dma_start(
                            out[img][:, done_rows:rows_ready, :],
                            ot3[:, done_rows:rows_ready, :],
                        )
                        done_rows = rows_ready
    return nc


def _prep_x(x, np_dt):
    """[N, 64, 112, 112] int -> [N, 128, FLAT] with shifted upper half."""
    n = x.shape[0]
    xc = np.clip(np.asarray(x), 0, 7).astype(np.float32).astype(np_dt)
    buf = np.zeros((n, C, FLAT + 112), dtype=np_dt)
    buf[:, :, : H * W] = xc.reshape(n, C, H * W)
    return np.ascontiguousarray(
        np.concatenate([buf[:, :, :FLAT], buf[:, :, 112 : 112 + FLAT]], axis=1)
    )


def _prep_w(w1, w2, w3, np_dt, use_fp8):
    """Assemble W7 and lay out the stationary (lhsT) weight tiles."""
    w1, w2, w3 = (np.asarray(w) for w in (w1, w2, w3))
    W7 = np.zeros((O, C, 7, 7), np.float32)
    W7[:, :, :, 0:3] += w1
    W7[:, :, :, 2:5] += w2
    W7[:, :, :, 4:7] += w3
    # T8[kh, kw, c, o], kh padded to 8 with zeros
    T8 = np.zeros((8, 7, C, O), np.float32)
    T8[:7] = W7.transpose(2, 3, 1, 0)
    if use_fp8:
        taps = TAPS_FP8
        wt = np.zeros((128, len(taps), 2, O), np.float32)
        for q, (kh0, kw) in enumerate(taps):
            for half in range(2):
                for i in range(2):
                    wt[half * 64 : half * 64 + 64, q, i] = T8[kh0 + half + 2 * i, kw]
        wt = wt.reshape(128, len(taps) * 2 * O)
    else:
        taps = TAPS_BF16
        wt = np.zeros((128, len(taps), O), np.float32)
        for q, (kh0, kw) in enumerate(taps):
            for half in range(2):
                wt[half * 64 : half * 64 + 64, q] = T8[kh0 + half, kw]
        wt = wt.reshape(128, len(taps) * O)
    return np.ascontiguousarray(wt.astype(np_dt))


_cached = {}


def _get_program(use_fp8):
    key = bool(use_fp8)
    if key not in _cached:
        nc = _build(use_fp8)
        nc.finalize()  # Bacc.compile(): sync legalization for TRN2
        _cached[key] = nc
    return _cached[key]


def kernel(x, w1, w2, w3):
    global last_exec_time_ns
    use_fp8 = USE_FP8
    np_dt = mybir.dt.np(mybir.dt.float8e4 if use_fp8 else mybir.dt.bfloat16)

    xp = _prep_x(x, np_dt)                    # [16, 128, FLAT]
    wt = _prep_w(w1, w2, w3, np_dt, use_fp8)  # [128, wcols]

    nc = _get_program(use_fp8)
    in_maps = [
        {"xp": xp[i * IMGS : (i + 1) * IMGS], "wt": wt} for i in range(N_CORES)
    ]
    res = run_bass_kernel_spmd(nc, in_maps, list(range(N_CORES)), trace=TRACE)
    last_exec_time_ns = res.exec_time_ns
    return np.concatenate([r["out"] for r in res.results], axis=0)


# revision 31
# speedup vs baseline: 1.0634x; 1.0634x over previous
"""Trainium2 Bass kernel: 7x7 VALID conv (expressed as three 7x3 convs).

x: [16, 64, 112, 112] int32 (values 0..7), w1/w2/w3: [128, 64, 7, 3]
(small ints 0..6; w2 only has its middle column nonzero).  The reference
out1[:, :, :, :-2] + out2[:, :, :, 2:-2] + out3[:, :, :, 2:] is exactly a
VALID 7x7 conv with W7 (w1 -> cols 0:3, w2 -> cols 2:5, w3 -> cols 4:7),
output [16, 128, 106, 106] float32.  All arithmetic is small-integer and
exactly representable in bf16/fp8e4m3 with fp32 PSUM accumulation, so the
kernel result is bit-exact vs the reference.

Strategy: data-parallel over batch N across 8 cores (2 images/core).
Direct conv as implicit GEMM on the TensorEngine:
  - SBUF x layout [128, 113*112]: partitions 0:64 = channels (flat H*W,
    one zero pad row), partitions 64:128 = same shifted by one image row
    (+112 elements).  A K=128 matmul therefore contracts 64 channels x 2
    vertical taps (kh, kh+1) at once.
  - fp8 DoubleRow additionally contracts 2 sub-elements per partition via
    a [128, 2, N] rhs AP with pair-stride 224 (= 2 rows): 4 vertical taps
    per matmul -> 14 matmuls per output tile instead of 28.
  - Output tiles are 4 output rows x 112 columns (N=448, only 106 cols
    valid per row); invalid columns are computed and discarded by the
    strided PSUM->DRAM store.
"""

import numpy as np

import concourse.bass as bass
import concourse.mybir as mybir
from concourse import tile
from concourse.ap import AP
from concourse.bacc import Bacc
from concourse.bass_utils import run_bass_kernel_spmd
from concourse.tile import add_dep_helper

N_CORES = 8
IMGS = 2            # images per core
C, O = 64, 128
H = W = 112
HO = WO = 106
FLAT = 113 * 112    # per-image padded flat length (one zero row at the end)
R = 4               # output rows per tile

USE_FP8 = True
TRACE = False  # set True (e.g. from test.py) to capture an NTFF profile

# tap schedules: (kh0, kw) covering kh0 + {0,1} (bf16) or kh0 + {0,1,2,3} (fp8)
TAPS_FP8 = [(kh0, kw) for kw in range(7) for kh0 in (0, 4)]     # 14 taps
TAPS_BF16 = [(kh0, kw) for kw in range(7) for kh0 in (0, 2, 4, 6)]  # 28 taps

GROUPS = [(h0, min(R, HO - h0)) for h0 in range(0, HO, R)]      # 27 groups

_FOLLOW_GROUPS = set()  # {(img, gi)}: debug — trace Tile dep insertion

# x-load chunk ends (flat elements): compute starts once the first chunk
# lands instead of waiting for the whole 12.4 KB/partition image
XCHUNKS = [1792, 5376, 9072, FLAT]
# output store boundaries (rows): tapered so the final DMA is small and
# the kernel-exit barrier isn't stuck behind a large transfer
OUT_ROWS = [36, 72, 100]

last_exec_time_ns = None


def _build(use_fp8, imgs=IMGS, groups=GROUPS):
    dt = mybir.dt.float8e4 if use_fp8 else mybir.dt.bfloat16
    taps = TAPS_FP8 if use_fp8 else TAPS_BF16
    wcols = len(taps) * (2 * 128 if use_fp8 else 128)

    # Bacc (not bare Bass): its finalize() legalizes sync for TRN2 —
    # splits >1-wait instructions into EventSemaphore chains and moves
    # matmul waits onto ldweights
    nc = Bacc()
    xp = nc.declare_dram_parameter("xp", [imgs, 128, FLAT], dt, isOutput=False)
    wt = nc.declare_dram_parameter("wt", [128, wcols], dt, isOutput=False)
    out = nc.declare_dram_parameter(
        "out", [imgs, O, HO, WO], mybir.dt.float32, isOutput=True
    )

    NPS = 6  # PSUM banks cycled round-robin (of 8)

    with tile.TileContext(nc) as tc:
        with (
            tc.tile_pool(name="xpool", bufs=2) as xpool,
            tc.tile_pool(name="wpool", bufs=1) as wpool,
            tc.tile_pool(name="opool", bufs=2) as opool,
            tc.tile_pool(name="psum", bufs=1, space="PSUM") as psum_pool,
        ):
            wtile = wpool.tile([128, wcols], dt)
            nc.sync.dma_start(wtile[:], wt[:])
            if use_fp8:
                w4 = wtile.rearrange("p (q i m) -> p q i m", i=2, m=128)
            else:
                w3 = wtile.rearrange("p (q m) -> p q m", m=128)

            # walrus only allows ONE sync-wait on a Matmult.  Tile's slot
            # allocator makes the first matmul of a group that reuses a PSUM
            # bank wait on the bank's release (PE sem AND the DVE copy's sem
            # = 2 waits).  Pre-advance the PE engine's observed DVE/DMA
            # clocks instead: hang sync deps on the PREVIOUS group's last
            # matmuls (0 waits, spare budget); wait emission is clock-aware,
            # so the reusing matmul then only emits its same-engine PE wait.
            slot_free = {}  # psum tag -> copy instruction that freed the slot
            prev_mms = None  # previous group's matmul instructions

            for img in range(imgs):
                xt = xpool.tile([128, FLAT], dt, tag="x")
                xdmas = []
                a = 0
                for end in XCHUNKS:
                    xdmas.append(nc.sync.dma_start(xt[:, a:end], xp[img][:, a:end]))
                    a = end
                xfull = xt[:]
                pdim = list(xfull.ap[0])
                # whole-image staging buffer; per-group slices are written
                # once each, so the DVE copies carry no WAR-on-DMA waits
                ot = opool.tile([128, HO * WO], mybir.dt.float32, tag="ot")
                ot3 = ot.rearrange("p (h w) -> p h w", w=WO)
                done_rows = 0
                have_chunk = -1
                bounds = [b for b in OUT_ROWS if b < groups[-1][0] + groups[-1][1]]
                bounds.append(groups[-1][0] + groups[-1][1])

                for gi, (h0, r) in enumerate(groups):
                    n = r * 112
                    tag = f"ps{gi % NPS}"
                    # full-bank allocation (512 f32 = 2 KB): unaligned PSUM
                    # tiles share banks and start=True's bank-granular clear
                    # then drags in cross-tile deps
                    psb = psum_pool.tile(
                        [128, 512], mybir.dt.float32, tag=tag, bufs=1
                    )
                    ps = psb[:, 0:n]
                    pre_deps = []
                    if tag in slot_free:
                        pre_deps.append(slot_free[tag])
                    # highest flat index this group's matmuls read
                    max_flat = min((h0 + 4) * 112 + 678, FLAT)
                    while (
                        have_chunk < 0 or XCHUNKS[have_chunk] < max_flat
                    ) and have_chunk < len(XCHUNKS) - 1:
                        have_chunk += 1
                        pre_deps.append(xdmas[have_chunk].ins)
                    if prev_mms is not None:
                        for k, d in enumerate(pre_deps):
                            add_dep_helper(
                                prev_mms[-1 - k].ins, d,
                                reason="pre-advance PE observed clock",
                            )
                    mms = []
                    for i, (kh0, kw) in enumerate(taps):
                        f0 = (h0 + kh0) * 112 + kw
                        if use_fp8:
                            rhs = AP(
                                xfull.tensor,
                                xfull.offset + f0,
                                [pdim, [224, 2], [1, n]],
                            )
                            mm = nc.tensor.matmul(
                                ps[:],
                                w4[:, i],
                                rhs,
                                start=(i == 0),
                                stop=(i == len(taps) - 1),
                                perf_mode=mybir.MatmulPerfMode.DoubleRow,
                            )
                        else:
                            rhs = AP(
                                xfull.tensor,
                                xfull.offset + f0,
                                [pdim, [1, n]],
                            )
                            mm = nc.tensor.matmul(
                                ps[:],
                                w3[:, i],
                                rhs,
                                start=(i == 0),
                                stop=(i == len(taps) - 1),
                            )
                        mms.append(mm)
                        if _FOLLOW_GROUPS and (img, gi) in _FOLLOW_GROUPS:
                            tile.tile_follow(mm, log_all_deps=True)
                    prev_mms = mms
                    ps3 = ps.rearrange("p (r w) -> p r w", w=112)
                    cp = nc.vector.tensor_copy(ot3[:, h0 : h0 + r, :], ps3[:, :, 0:WO])
                    if _FOLLOW_GROUPS and (img, gi) in _FOLLOW_GROUPS:
                        tile.tile_follow(cp, log_all_deps=True)
                    slot_free[tag] = cp.ins
                    # drain finished rows in tapered chunks so the final DMA
                    # before the exit barrier is small
                    rows_ready = h0 + r
                    if bounds and rows_ready >= bounds[0]:
                        while bounds and rows_ready >= bounds[0]:
                            bounds.pop(0)
                        nc.gpsimd.dma_start(
                            out[img][:, done_rows:rows_ready, :],
                            ot3[:, done_rows:rows_ready, :],
                        )
                        done_rows = rows_ready
    return nc


def _prep_x(x, np_dt):
    """[N, 64, 112, 112] int -> [N, 128, FLAT] with shifted upper half."""
    n = x.shape[0]
    xc = np.clip(np.asarray(x), 0, 7).astype(np.float32).astype(np_dt)
    buf = np.zeros((n, C, FLAT + 112), dtype=np_dt)
    buf[:, :, : H * W] = xc.reshape(n, C, H * W)
    return np.ascontiguousarray(
        np.concatenate([buf[:, :, :FLAT], buf[:, :, 112 : 112 + FLAT]], axis=1)
    )


def _prep_w(w1, w2, w3, np_dt, use_fp8):
    """Assemble W7 and lay out the stationary (lhsT) weight tiles."""
    w1, w2, w3 = (np.asarray(w) for w in (w1, w2, w3))
    W7 = np.zeros((O, C, 7, 7), np.float32)
    W7[:, :, :, 0:3] += w1
    W7[:, :, :, 2:5] += w2
    W7[:, :, :, 4:7] += w3
    # T8[kh, kw, c, o], kh padded to 8 with zeros
    T8 = np.zeros((8, 7, C, O), np.float32)
    T8[:7] = W7.transpose(2, 3, 1, 0)
    if use_fp8:
        taps = TAPS_FP8
        wt = np.zeros((128, len(taps), 2, O), np.float32)
        for q, (kh0, kw) in enumerate(taps):
            for half in range(2):
                for i in range(2):
                    wt[half * 64 : half * 64 + 64, q, i] = T8[kh0 + half + 2 * i, kw]
        wt = wt.reshape(128, len(taps) * 2 * O)
    else:
        taps = TAPS_BF16
        wt = np.zeros((128, len(taps), O), np.float32)
        for q, (kh0, kw) in enumerate(taps):
            for half in range(2):
                wt[half * 64 : half * 64 + 64, q] = T8[kh0 + half, kw]
        wt = wt.reshape(128, len(taps) * O)
    return np.ascontiguousarray(wt.astype(np_dt))


_cached = {}


def _get_program(use_fp8):
    key = bool(use_fp8)
    if key not in _cached:
        nc = _build(use_fp8)
        nc.finalize()  # Bacc.compile(): sync legalization for TRN2
        _cached[key] = nc
    return _cached[key]


def kernel(x, w1, w2, w3):
    global last_exec_time_ns
    use_fp8 = USE_FP8
    np_dt = mybir.dt.np(mybir.dt.float8e4 if use_fp8 else mybir.dt.bfloat16)

    xp = _prep_x(x, np_dt)                    # [16, 128, FLAT]
    wt = _prep_w(w1, w2, w3, np_dt, use_fp8)  # [128, wcols]

    nc = _get_program(use_fp8)
    in_maps = [
        {"xp": xp[i * IMGS : (i + 1) * IMGS], "wt": wt} for i in range(N_CORES)
    ]
    res = run_bass_kernel_spmd(nc, in_maps, list(range(N_CORES)), trace=TRACE)
    last_exec_time_ns = res.exec_time_ns
    return np.concatenate([r["out"] for r in res.results], axis=0)
